# revision 8
# baseline (speedup 1.0000x reference)
"""TRN2 Bass kernel for nn_DQN (topk_masking).

reference:
    h = relu(x @ W1 + b1); h = relu(h @ W2 + b2); logits = h @ W3 + b3
    mask[b, possible_moves[b, :]] = 1
    out = softmax(logits * mask, axis=1)

Strategy (8 NeuronCores, data-parallel over batch, 2048 rows/core):
  - host: transpose x -> xT [128, B]; fold b2/b3 into augmented weight rows.
  - PE: h1T/h2T computed transposed ([hid, batch]) so W1/W2aug are the
    stationary operands; logits via lhsT = h2aug columns (K=25 incl. ones row
    so b3 is free), rhs = W3aug slices; [128, 512] PSUM chunks.
  - GPSIMD local_scatter builds the 0/1 mask [128 rows, 4096] bf16 from
    possible_moves (3 scatters of index ranges [0,2046)/[2046,4092)/[4092,4096)
    since local_scatter caps num_elems at 2046; DVE computes the per-range
    poisoned indices: out-of-range -> negative = ignored by the ucode).
  - DVE: filtered = logits * mask (fp32 x bf16).
  - ACT: E = exp(filtered) with accum_out giving the row sum Z directly
    (illegal positions contribute exp(0)=1 exactly as the reference does).
  - out = E * (1/Z): per-partition tensor_scalar, split DVE/ACT for balance.

reps>1 wraps the main loop in a dynamic For_i purely for timing experiments
(single NEFF call amortizes the host->device roundtrip over reps passes).
"""

import os
import sys

import numpy as np

for _p in ("/root/.axon_site", "/root/.axon_site/_ro/trn_rl_repo",
           "/root/.axon_site/_ro/pypackages"):
    if os.path.isdir(_p) and _p not in sys.path:
        sys.path.append(_p)

B, IN_DIM, HID, OUT_DIM, K = 16384, 128, 24, 4096, 256
NCORES = 8
BS = B // NCORES          # 2048 rows per core
NT = BS // 128            # 16 tiles of 128 rows
HAUG = HID + 1            # 25: hidden + ones row

_cache = {}


def _build_nc(reps=1):
    import concourse.bacc as bacc
    import concourse.mybir as mybir
    import concourse.tile as tile

    F32 = mybir.dt.float32
    BF16 = mybir.dt.bfloat16
    I32 = mybir.dt.int32
    I16 = mybir.dt.int16
    ALU = mybir.AluOpType
    ACTF = mybir.ActivationFunctionType

    nc = bacc.Bacc("TRN2", target_bir_lowering=False, debug=False,
                   num_devices=NCORES)

    xT = nc.dram_tensor("xT", [IN_DIM, BS], F32, kind="ExternalInput").ap()
    pm = nc.dram_tensor("pm", [BS, K], I32, kind="ExternalInput").ap()
    w1 = nc.dram_tensor("w1", [IN_DIM, HID], F32, kind="ExternalInput").ap()
    b1 = nc.dram_tensor("b1", [HID, 1], F32, kind="ExternalInput").ap()
    w2a = nc.dram_tensor("w2a", [HAUG, HID], F32, kind="ExternalInput").ap()
    w3a = nc.dram_tensor("w3a", [HAUG, OUT_DIM], F32, kind="ExternalInput").ap()
    out = nc.dram_tensor("out", [BS, OUT_DIM], F32, kind="ExternalOutput").ap()

    with tile.TileContext(nc) as tc:
        with tc.tile_pool(name="singles", bufs=1) as singles:
            xT_s = singles.tile([IN_DIM, BS], F32)
            nc.sync.dma_start(out=xT_s, in_=xT)
            w1_s = singles.tile([IN_DIM, HID], F32)
            nc.sync.dma_start(out=w1_s, in_=w1)
            b1_s = singles.tile([HID, 1], F32)
            nc.sync.dma_start(out=b1_s, in_=b1)
            w2a_s = singles.tile([HAUG, HID], F32)
            nc.sync.dma_start(out=w2a_s, in_=w2a)
            w3a_s = singles.tile([HAUG, OUT_DIM], F32)
            nc.sync.dma_start(out=w3a_s, in_=w3a)
            ones_s = singles.tile([128, K], BF16)
            nc.vector.memset(ones_s, 1.0)
            # ones row (partition 24) can't be memset alone: engine base
            # partition must be 0/32/64/96 -> memset whole tile, relu
            # overwrites rows 0..23
            h2a_s = singles.tile([HAUG, BS], F32)
            nc.vector.memset(h2a_s, 1.0)

            # ---- tiny MLP: h2aug [25, BS], computed in 512-col chunks ----
            with tc.tile_pool(name="mlp_ps", bufs=2, space="PSUM") as mlp_ps, \
                 tc.tile_pool(name="mlp", bufs=2) as mlp:
                for c in range(BS // 512):
                    sl = slice(c * 512, (c + 1) * 512)
                    p1 = mlp_ps.tile([HID, 512], F32, tag="p1")
                    nc.tensor.matmul(p1, w1_s, xT_s[:, sl], start=True,
                                     stop=True)
                    h1a = mlp.tile([HAUG, 512], F32, tag="h1")
                    nc.vector.memset(h1a, 1.0)
                    nc.scalar.activation(h1a[0:HID, :], p1, ACTF.Relu,
                                         bias=b1_s)
                    p2 = mlp_ps.tile([HID, 512], F32, tag="p2")
                    nc.tensor.matmul(p2, w2a_s, h1a, start=True, stop=True)
                    nc.scalar.activation(h2a_s[0:HID, sl], p2, ACTF.Relu)

            # ---- main loop over 16 tiles of 128 batch rows ----
            with tc.tile_pool(name="io", bufs=3) as iop, \
                 tc.tile_pool(name="idx", bufs=3) as idxp, \
                 tc.tile_pool(name="mask", bufs=2) as maskp, \
                 tc.tile_pool(name="big", bufs=3) as bigp, \
                 tc.tile_pool(name="epool", bufs=4) as epool, \
                 tc.tile_pool(name="outp", bufs=2) as outp, \
                 tc.tile_pool(name="ps", bufs=2, space="PSUM") as psp, \
                 tc.tile_pool(name="small", bufs=4) as smallp:

                def tile_body(t):
                    rows = slice(t * 128, (t + 1) * 128)

                    pm_s = iop.tile([128, K], I32, tag="pm", name="pm_s")
                    nc.scalar.dma_start(out=pm_s, in_=pm[rows, :])

                    # poisoned per-range indices; negatives are ignored by
                    # the scatter ucode, so only too-high values need help
                    v0 = idxp.tile([128, K], I16, tag="v0", name="v0")
                    nc.vector.tensor_scalar(v0, pm_s, 2046, None, ALU.is_lt)
                    idx0 = idxp.tile([128, K], I16, tag="i0", name="idx0")
                    nc.vector.affine_then_add(idx0, v0, pm_s, 4096.0, -4096.0)
                    v1 = idxp.tile([128, K], I16, tag="v1", name="v1")
                    nc.vector.tensor_scalar(v1, pm_s, 4092, None, ALU.is_lt)
                    idx1 = idxp.tile([128, K], I16, tag="i1", name="idx1")
                    nc.vector.affine_then_add(idx1, v1, pm_s, 2050.0, -4096.0)
                    idx2 = idxp.tile([128, K], I16, tag="i2", name="idx2")
                    nc.vector.tensor_scalar(idx2, pm_s, 4092, None,
                                            ALU.subtract)

                    m = maskp.tile([128, OUT_DIM], BF16, tag="m", name="m")
                    nc.gpsimd.local_scatter(m[:, 0:2046], ones_s, idx0,
                                            128, 2046, K)
                    nc.gpsimd.local_scatter(m[:, 2046:4092], ones_s, idx1,
                                            128, 2046, K)
                    nc.gpsimd.local_scatter(m[:, 4092:4096], ones_s, idx2,
                                            128, 4, K)

                    out_t = outp.tile([128, OUT_DIM], F32, tag="out",
                                      name="out_t")
                    e_tiles = []
                    z_tiles = []
                    for h in range(2):
                        hsl = slice(h * 2048, (h + 1) * 2048)
                        pl = psp.tile([128, 2048], F32, tag="pl", name="pl")
                        for n in range(4):
                            ns = h * 2048 + n * 512
                            nc.tensor.matmul(
                                pl[:, n * 512:(n + 1) * 512],
                                h2a_s[:, rows],
                                w3a_s[:, ns:ns + 512],
                                start=True, stop=True,
                            )
                        filt = bigp.tile([128, 2048], F32, tag="filt",
                                         name="filt")
                        nc.vector.tensor_tensor(
                            out=filt, in0=pl, in1=m[:, hsl], op=ALU.mult)
                        e = epool.tile([128, 2048], F32, tag="e", name="e")
                        zp = smallp.tile([128, 1], F32, tag=f"zp{h}",
                                         name=f"zp{h}")
                        nc.scalar.activation(e, filt, ACTF.Exp, accum_out=zp)
                        e_tiles.append(e)
                        z_tiles.append(zp)

                    z = smallp.tile([128, 1], F32, tag="z", name="z")
                    nc.vector.tensor_tensor(out=z, in0=z_tiles[0],
                                            in1=z_tiles[1], op=ALU.add)
                    invz = smallp.tile([128, 1], F32, tag="invz", name="invz")
                    nc.vector.reciprocal(invz, z)

                    # normalize: half on DVE (2x fp32), half on ACT
                    nc.vector.tensor_scalar(out_t[:, 0:2048], e_tiles[0],
                                            invz, None, ALU.mult)
                    nc.scalar.mul(out_t[:, 2048:4096], e_tiles[1], invz)

                    nc.sync.dma_start(out=out[rows, :], in_=out_t)

                if reps == 1:
                    for t in range(NT):
                        tile_body(t)
                else:
                    with tc.For_i(0, reps, 1):
                        for t in range(NT):
                            tile_body(t)

    nc.compile()
    return nc


def _get_nc(reps=1):
    key = f"nc{reps}"
    if key not in _cache:
        _cache[key] = _build_nc(reps)
    return _cache[key]


def _prep_inputs(x, possible_moves, W1, b1, W2, b2, W3, b3):
    x = np.ascontiguousarray(np.asarray(x, dtype=np.float32))
    pm = np.ascontiguousarray(np.asarray(possible_moves).astype(np.int32))
    W1 = np.ascontiguousarray(np.asarray(W1, dtype=np.float32))
    b1c = np.asarray(b1, dtype=np.float32).reshape(HID, 1)
    w2a = np.ascontiguousarray(
        np.concatenate([np.asarray(W2, np.float32),
                        np.asarray(b2, np.float32)[None, :]], axis=0))
    w3a = np.ascontiguousarray(
        np.concatenate([np.asarray(W3, np.float32),
                        np.asarray(b3, np.float32)[None, :]], axis=0))
    xT = np.ascontiguousarray(x.T)  # [IN_DIM, B]

    in_maps = []
    for c in range(NCORES):
        sl = slice(c * BS, (c + 1) * BS)
        in_maps.append({
            "xT": np.ascontiguousarray(xT[:, sl]),
            "pm": np.ascontiguousarray(pm[sl, :]),
            "w1": W1,
            "b1": b1c,
            "w2a": w2a,
            "w3a": w3a,
        })
    return in_maps


def kernel(x, possible_moves, W1, b1, W2, b2, W3, b3):
    from concourse.bass_utils import run_bass_kernel_spmd

    in_maps = _prep_inputs(x, possible_moves, W1, b1, W2, b2, W3, b3)
    nc = _get_nc()
    res = run_bass_kernel_spmd(nc, in_maps, core_ids=list(range(NCORES)))
    return np.concatenate([res.results[c]["out"] for c in range(NCORES)],
                          axis=0)


# revision 28
# speedup vs baseline: 3.8581x; 3.8581x over previous
"""TRN2 Bass kernel for nn_DQN (topk_masking).

reference:
    h = relu(x @ W1 + b1); h = relu(h @ W2 + b2); logits = h @ W3 + b3
    mask[b, possible_moves[b, :]] = 1
    out = softmax(logits * mask, axis=1)

Strategy (8 NeuronCores, data-parallel over batch, 2048 rows/core):
  - host: transpose x -> xT [128, B]; fold b2/b3 into augmented weight rows.
  - PE: h1T/h2T computed transposed ([hid, batch]) so W1/W2aug are the
    stationary operands; logits via lhsT = h2aug columns (K=25 incl. ones row
    so b3 is free), rhs = W3aug slices; [128, 512] PSUM chunks.
  - GPSIMD local_scatter builds the 0/1 mask [128 rows, 4096] bf16 from
    possible_moves (3 scatters of index ranges [0,2046)/[2046,4092)/[4092,4096)
    since local_scatter caps num_elems at 2046; DVE computes the per-range
    poisoned indices: out-of-range -> negative = ignored by the ucode).
  - DVE: filtered = logits * mask (fp32 x bf16).
  - ACT: E = exp(filtered) with accum_out giving the row sum Z directly
    (illegal positions contribute exp(0)=1 exactly as the reference does).
  - out = E * (1/Z): per-partition tensor_scalar, split DVE/ACT for balance.

reps>1 wraps the main loop in a dynamic For_i purely for timing experiments
(single NEFF call amortizes the host->device roundtrip over reps passes).
"""

import os
import sys

import numpy as np

for _p in ("/root/.axon_site", "/root/.axon_site/_ro/trn_rl_repo",
           "/root/.axon_site/_ro/pypackages"):
    if os.path.isdir(_p) and _p not in sys.path:
        sys.path.append(_p)

B, IN_DIM, HID, OUT_DIM, K = 16384, 128, 24, 4096, 256
NCORES = 8
BS = B // NCORES          # 2048 rows per core
NT = BS // 128            # 16 tiles of 128 rows
HAUG = HID + 1            # 25: hidden + ones row

_cache = {}

# matmul operand dtype: float32r is ~4x faster on the PE but rounds operands
# to ~TF32 precision (rel err ~1.3e-4 vs 8.3e-6). Default to exact fp32.
MM_F32R = False


def _build_nc(reps=1, variant="full"):
    import concourse.bacc as bacc
    import concourse.mybir as mybir
    import concourse.tile as tile

    F32 = mybir.dt.float32
    F32R = mybir.dt.float32r   # fp32 bits, PE runs 1 cyc/row (vs 4 for fp32)
    BF16 = mybir.dt.bfloat16
    I32 = mybir.dt.int32
    I16 = mybir.dt.int16
    ALU = mybir.AluOpType
    ACTF = mybir.ActivationFunctionType

    MMDT = F32R if MM_F32R else F32

    nc = bacc.Bacc("TRN2", target_bir_lowering=False, debug=False,
                   num_devices=NCORES)

    xT = nc.dram_tensor("xT", [IN_DIM, BS], MMDT, kind="ExternalInput").ap()
    pm = nc.dram_tensor("pm", [BS, K], I32, kind="ExternalInput").ap()
    w1 = nc.dram_tensor("w1", [IN_DIM, HID], MMDT, kind="ExternalInput").ap()
    b1 = nc.dram_tensor("b1", [HID, 1], F32, kind="ExternalInput").ap()
    w2a = nc.dram_tensor("w2a", [HAUG, HID], MMDT, kind="ExternalInput").ap()
    w3a = nc.dram_tensor("w3a", [HAUG, OUT_DIM], MMDT, kind="ExternalInput").ap()
    onesd = nc.dram_tensor("onesd", [1, BS], MMDT, kind="ExternalInput").ap()
    out = nc.dram_tensor("out", [BS, OUT_DIM], F32, kind="ExternalOutput").ap()

    GRP = 4           # tiles per transform group
    NGRP = NT // GRP

    with tile.TileContext(nc) as tc:
        with tc.tile_pool(name="singles", bufs=1) as singles:
            do_scatter = variant in ("full", "noexp", "noskew")
            pmv = pm.rearrange("(t p) k -> p t k", p=128)
            if do_scatter:
                # index prep first: pm chunks on the ACT HWDGE ring (separate
                # FIFO from the weight loads below), transforms per chunk so
                # GPSIMD scatters start almost immediately
                idx0_a = singles.tile([128, NT, K], I16)
                idx1_a = singles.tile([128, NT, K], I16)
                idx2_a = singles.tile([128, NT, K], I16)
                with tc.tile_pool(name="pmp", bufs=2) as pmpool:
                    for g in range(NGRP):
                        gs = slice(g * GRP, (g + 1) * GRP)
                        pm_g = pmpool.tile([128, GRP, K], I32, tag="pm",
                                           name="pm_g")
                        nc.scalar.dma_start(out=pm_g, in_=pmv[:, gs, :])
                        v0 = pmpool.tile([128, GRP, K], I16, tag="v0",
                                         name="v0")
                        nc.vector.tensor_scalar(v0, pm_g, 2046, None,
                                                ALU.is_lt)
                        nc.vector.affine_then_add(idx0_a[:, gs, :], v0,
                                                  pm_g, 4096.0, -4096.0)
                        v1 = pmpool.tile([128, GRP, K], I16, tag="v1",
                                         name="v1")
                        nc.vector.tensor_scalar(v1, pm_g, 4092, None,
                                                ALU.is_lt)
                        nc.vector.affine_then_add(idx1_a[:, gs, :], v1,
                                                  pm_g, 2050.0, -4096.0)
                        nc.vector.tensor_scalar(idx2_a[:, gs, :], pm_g,
                                                4092, None, ALU.subtract)

            xT_s = singles.tile([IN_DIM, BS], MMDT)
            nc.sync.dma_start(out=xT_s, in_=xT)
            w1_s = singles.tile([IN_DIM, HID], MMDT)
            nc.sync.dma_start(out=w1_s, in_=w1)
            b1_s = singles.tile([HID, 1], F32)
            nc.sync.dma_start(out=b1_s, in_=b1)
            w2a_s = singles.tile([HAUG, HID], MMDT)
            nc.sync.dma_start(out=w2a_s, in_=w2a)
            w3a_s = singles.tile([HAUG, OUT_DIM], MMDT)
            nc.sync.dma_start(out=w3a_s, in_=w3a)
            ones_s = singles.tile([128, K], BF16)
            nc.vector.memset(ones_s, 1.0)
            # ones row (partition 24): engine ops can't target base
            # partition 24 and memset can't produce f32r, so DMA the row
            # from a host-side ones constant; relu writes rows 0..23
            h2a_s = singles.tile([HAUG, BS], MMDT)
            nc.sync.dma_start(out=h2a_s[HID:HAUG, :], in_=onesd)

            # ---- tiny MLP: h2aug [25, BS], computed in 512-col chunks ----
            with tc.tile_pool(name="mlp_ps", bufs=2, space="PSUM") as mlp_ps, \
                 tc.tile_pool(name="mlp", bufs=2) as mlp:
                for c in range(BS // 512):
                    sl = slice(c * 512, (c + 1) * 512)
                    p1 = mlp_ps.tile([HID, 512], F32, tag="p1")
                    nc.tensor.matmul(p1, w1_s, xT_s[:, sl], start=True,
                                     stop=True)
                    h1a = mlp.tile([HAUG, 512], MMDT, tag="h1")
                    nc.sync.dma_start(out=h1a[HID:HAUG, :],
                                      in_=onesd[:, 0:512])
                    nc.scalar.activation(h1a[0:HID, :], p1, ACTF.Relu,
                                         bias=b1_s)
                    p2 = mlp_ps.tile([HID, 512], F32, tag="p2")
                    nc.tensor.matmul(p2, w2a_s, h1a, start=True,
                                     stop=True)
                    nc.scalar.activation(h2a_s[0:HID, sl], p2, ACTF.Relu)

            # ---- main loop over 16 tiles of 128 batch rows ----
            with tc.tile_pool(name="mask", bufs=3) as maskp, \
                 tc.tile_pool(name="big", bufs=4) as bigp, \
                 tc.tile_pool(name="epool", bufs=5) as epool, \
                 tc.tile_pool(name="outp", bufs=2) as outp, \
                 tc.tile_pool(name="ps", bufs=4, space="PSUM") as psp, \
                 tc.tile_pool(name="small", bufs=4) as smallp:

                def compute_phase(t):
                    """scatter + matmuls + mask-mult + exp for tile t;
                    returns (e_tiles, z_tiles) to normalize later."""
                    rows = slice(t * 128, (t + 1) * 128)

                    m = maskp.tile([128, OUT_DIM], BF16, tag="m", name="m")
                    if do_scatter:
                        nc.gpsimd.local_scatter(m[:, 0:2046], ones_s,
                                                idx0_a[:, t, :], 128, 2046, K)
                        nc.gpsimd.local_scatter(m[:, 2046:4092], ones_s,
                                                idx1_a[:, t, :], 128, 2046, K)
                        nc.gpsimd.local_scatter(m[:, 4092:4096], ones_s,
                                                idx2_a[:, t, :], 128, 4, K)
                    else:
                        nc.vector.memset(m, 1.0)

                    e_tiles = []
                    z_tiles = []
                    for h in range(2):
                        filt = bigp.tile([128, 2048], F32, tag="filt",
                                         name="filt")
                        for q in range(2):
                            pl = psp.tile([128, 1024], F32, tag="pl",
                                          name="pl")
                            for n in range(2):
                                ns = h * 2048 + q * 1024 + n * 512
                                nc.tensor.matmul(
                                    pl[:, n * 512:(n + 1) * 512],
                                    h2a_s[:, rows],
                                    w3a_s[:, ns:ns + 512],
                                    start=True, stop=True,
                                )
                            if variant == "nomask":
                                nc.vector.tensor_copy(
                                    filt[:, q * 1024:(q + 1) * 1024], pl)
                            else:
                                nc.vector.tensor_tensor(
                                    out=filt[:, q * 1024:(q + 1) * 1024],
                                    in0=pl,
                                    in1=m[:, h * 2048 + q * 1024:
                                          h * 2048 + (q + 1) * 1024],
                                    op=ALU.mult)
                        e = epool.tile([128, 2048], F32, tag="e", name="e")
                        zp = smallp.tile([128, 1], F32, tag=f"zp{h}",
                                         name=f"zp{h}")
                        if variant == "noexp":
                            nc.scalar.activation(e, filt, ACTF.Copy)
                            nc.vector.memset(zp, 1.0)
                        else:
                            nc.scalar.activation(e, filt, ACTF.Exp,
                                                 accum_out=zp)
                        e_tiles.append(e)
                        z_tiles.append(zp)
                    return e_tiles, z_tiles

                def norm_phase(t, e_tiles, z_tiles):
                    """1/Z + scale + store for tile t (skewed one tile back
                    so neither DVE nor ACT stalls on the other's fresh
                    output)."""
                    rows = slice(t * 128, (t + 1) * 128)
                    out_t = outp.tile([128, OUT_DIM], F32, tag="out",
                                      name="out_t")
                    z = smallp.tile([128, 1], F32, tag="z", name="z")
                    nc.vector.tensor_tensor(out=z, in0=z_tiles[0],
                                            in1=z_tiles[1], op=ALU.add)
                    invz = smallp.tile([128, 1], F32, tag="invz", name="invz")
                    nc.vector.reciprocal(invz, z)
                    # normalize: half on DVE (2x fp32), half on ACT
                    nc.vector.tensor_scalar(out_t[:, 0:2048], e_tiles[0],
                                            invz, None, ALU.mult)
                    nc.scalar.mul(out_t[:, 2048:4096], e_tiles[1], invz)
                    nc.sync.dma_start(out=out[rows, :], in_=out_t)

                def main_loop():
                    if variant == "noskew":
                        for t in range(NT):
                            norm_phase(t, *compute_phase(t))
                        return
                    pending = None
                    for t in range(NT):
                        ez = compute_phase(t)
                        if pending is not None:
                            norm_phase(t - 1, *pending)
                        pending = ez
                    norm_phase(NT - 1, *pending)

                if reps == 1:
                    main_loop()
                else:
                    with tc.For_i(0, reps, 1):
                        main_loop()

    nc.compile()
    return nc


def _get_nc(reps=1, variant="full"):
    key = f"nc{reps}-{variant}"
    if key not in _cache:
        _cache[key] = _build_nc(reps, variant)
    return _cache[key]


def _prep_inputs(x, possible_moves, W1, b1, W2, b2, W3, b3):
    x = np.ascontiguousarray(np.asarray(x, dtype=np.float32))
    pm = np.ascontiguousarray(np.asarray(possible_moves).astype(np.int32))
    W1 = np.ascontiguousarray(np.asarray(W1, dtype=np.float32))
    b1c = np.asarray(b1, dtype=np.float32).reshape(HID, 1)
    w2a = np.ascontiguousarray(
        np.concatenate([np.asarray(W2, np.float32),
                        np.asarray(b2, np.float32)[None, :]], axis=0))
    w3a = np.ascontiguousarray(
        np.concatenate([np.asarray(W3, np.float32),
                        np.asarray(b3, np.float32)[None, :]], axis=0))
    xT = np.ascontiguousarray(x.T)  # [IN_DIM, B]
    ones_row = np.ones((1, BS), np.float32)

    in_maps = []
    for c in range(NCORES):
        sl = slice(c * BS, (c + 1) * BS)
        in_maps.append({
            "xT": np.ascontiguousarray(xT[:, sl]),
            "pm": np.ascontiguousarray(pm[sl, :]),
            "w1": W1,
            "b1": b1c,
            "w2a": w2a,
            "w3a": w3a,
            "onesd": ones_row,
        })
    return in_maps


def kernel(x, possible_moves, W1, b1, W2, b2, W3, b3):
    from concourse.bass_utils import run_bass_kernel_spmd

    in_maps = _prep_inputs(x, possible_moves, W1, b1, W2, b2, W3, b3)
    nc = _get_nc()
    res = run_bass_kernel_spmd(nc, in_maps, core_ids=list(range(NCORES)))
    return np.concatenate([res.results[c]["out"] for c in range(NCORES)],
                          axis=0)


# revision 35
# speedup vs baseline: 5.0567x; 1.3107x over previous
"""TRN2 Bass kernel for nn_DQN (topk_masking).

reference:
    h = relu(x @ W1 + b1); h = relu(h @ W2 + b2); logits = h @ W3 + b3
    mask[b, possible_moves[b, :]] = 1
    out = softmax(logits * mask, axis=1)

Strategy (8 NeuronCores, data-parallel over batch, 2048 rows/core):
  - host: transpose x -> xT [128, B]; fold b2/b3 into augmented weight rows.
  - index prep first: pm chunks + 5 batched DVE transforms per chunk so the
    GPSIMD scatter stream starts within a few us and never waits on DVE.
  - tiny MLP on PE in fp32 (cheap); the big logits matmul runs as float32r
    (1 cyc/row vs 4 for fp32) with f32r *residual correction passes*
    accumulated in PSUM: logits = h2_r @ W3_r (+ h2_r @ W3res_r + h2res_r @
    W3_r per MM_MODE), where X_r = round_f32r(X) comes from the engines' own
    f32r write-rounding and Xres_r = round_f32r(X - X_r).
  - GPSIMD local_scatter builds the 0/1 mask [128 rows, 4096] bf16 per tile
    (3 scatters: [0,2046)/[2046,4092)/[4092,4096), num_elems cap is 2046;
    out-of-range indices are made negative = ignored by the ucode).
  - DVE: filtered = logits * mask; ACT: E = exp(filtered) with accum_out
    giving the row-sum Z directly (illegal positions contribute exp(0)=1,
    matching the reference); out = E * (1/Z) split DVE/ACT.

reps>1 wraps the main loop in a dynamic For_i purely for timing experiments.
"""

import os
import sys

import numpy as np

for _p in ("/root/.axon_site", "/root/.axon_site/_ro/trn_rl_repo",
           "/root/.axon_site/_ro/pypackages"):
    if os.path.isdir(_p) and _p not in sys.path:
        sys.path.append(_p)

B, IN_DIM, HID, OUT_DIM, K = 16384, 128, 24, 4096, 256
NCORES = 8
BS = B // NCORES          # 2048 rows per core
NT = BS // 128            # 16 tiles of 128 rows
HAUG = HID + 1            # 25: hidden + ones row

_cache = {}

# logits matmul precision mode:
#   "f32"         exact, 4 cyc/row on PE (slow)
#   "f32r"        single f32r pass (~1.3e-4 rel err)
#   "f32r_w3res"  f32r + W3-residual pass
#   "f32r_h2res"  f32r + h2-residual pass
#   "f32r_both"   f32r + both residual passes
MM_MODE = "f32r_both"


def _build_nc(reps=1, variant="full"):
    import concourse.bacc as bacc
    import concourse.mybir as mybir
    import concourse.tile as tile

    F32 = mybir.dt.float32
    F32R = mybir.dt.float32r
    BF16 = mybir.dt.bfloat16
    I32 = mybir.dt.int32
    I16 = mybir.dt.int16
    ALU = mybir.AluOpType
    ACTF = mybir.ActivationFunctionType

    nc = bacc.Bacc("TRN2", target_bir_lowering=False, debug=False,
                   num_devices=NCORES)

    xT = nc.dram_tensor("xT", [IN_DIM, BS], F32, kind="ExternalInput").ap()
    pm = nc.dram_tensor("pm", [BS, K], I32, kind="ExternalInput").ap()
    w1 = nc.dram_tensor("w1", [IN_DIM, HID], F32, kind="ExternalInput").ap()
    b1 = nc.dram_tensor("b1", [HID, 1], F32, kind="ExternalInput").ap()
    w2a = nc.dram_tensor("w2a", [HAUG, HID], F32, kind="ExternalInput").ap()
    w3a = nc.dram_tensor("w3a", [HAUG, OUT_DIM], F32,
                         kind="ExternalInput").ap()
    onesd = nc.dram_tensor("onesd", [1, BS], F32, kind="ExternalInput").ap()
    out = nc.dram_tensor("out", [BS, OUT_DIM], F32, kind="ExternalOutput").ap()

    GRP = 4           # tiles per index-transform group
    NGRP = NT // GRP
    mm_f32 = (MM_MODE == "f32")

    with tile.TileContext(nc) as tc:
        with tc.tile_pool(name="singles", bufs=1) as singles:
            do_scatter = variant in ("full", "noexp", "noskew", "scatteronly")

            pmv = pm.rearrange("(t p) k -> p t k", p=128)
            if do_scatter:
                idx0_a = singles.tile([128, NT, K], I16)
                idx1_a = singles.tile([128, NT, K], I16)
                idx2_a = singles.tile([128, NT, K], I16)
                with tc.tile_pool(name="pmp", bufs=2) as pmpool:
                    for g in range(NGRP):
                        gs = slice(g * GRP, (g + 1) * GRP)
                        pm_g = pmpool.tile([128, GRP, K], I32, tag="pm",
                                           name="pm_g")
                        nc.scalar.dma_start(out=pm_g, in_=pmv[:, gs, :])
                        v0 = pmpool.tile([128, GRP, K], I16, tag="v0",
                                         name="v0")
                        nc.vector.tensor_scalar(v0, pm_g, 2046, None,
                                                ALU.is_lt)
                        nc.vector.affine_then_add(idx0_a[:, gs, :], v0,
                                                  pm_g, 4096.0, -4096.0)
                        v1 = pmpool.tile([128, GRP, K], I16, tag="v1",
                                         name="v1")
                        nc.vector.tensor_scalar(v1, pm_g, 4092, None,
                                                ALU.is_lt)
                        nc.vector.affine_then_add(idx1_a[:, gs, :], v1,
                                                  pm_g, 2050.0, -4096.0)
                        nc.vector.tensor_scalar(idx2_a[:, gs, :], pm_g,
                                                4092, None, ALU.subtract)

            ones_s = singles.tile([128, K], BF16)
            nc.vector.memset(ones_s, 1.0)

            # setup pool: fp32 masters that die once the f32r copies exist
            # (in f32 mode h2a/w3a live on in singles instead)
            setup = tc.tile_pool(name="setup", bufs=1)
            setupp = setup.__enter__()
            big_pool = singles if mm_f32 else setupp
            xT_s = setupp.tile([IN_DIM, BS], F32, name="xT_s")
            nc.sync.dma_start(out=xT_s, in_=xT)
            w1_s = setupp.tile([IN_DIM, HID], F32, name="w1_s")
            nc.sync.dma_start(out=w1_s, in_=w1)
            b1_s = setupp.tile([HID, 1], F32, name="b1_s")
            nc.sync.dma_start(out=b1_s, in_=b1)
            w2a_s = setupp.tile([HAUG, HID], F32, name="w2a_s")
            nc.sync.dma_start(out=w2a_s, in_=w2a)
            w3a_s = big_pool.tile([HAUG, OUT_DIM], F32, name="w3a_s")
            nc.sync.dma_start(out=w3a_s, in_=w3a)
            # ones row (partition 24): engines can't target base partition 24
            # alone -> DMA the row from a host-side ones constant; relu
            # writes rows 0..23
            h2a_s = big_pool.tile([HAUG, BS], F32, name="h2a_s")
            nc.sync.dma_start(out=h2a_s[HID:HAUG, :], in_=onesd)

            # ---- tiny MLP (fp32): h2aug [25, BS] in 512-col chunks ----
            with tc.tile_pool(name="mlp_ps", bufs=2, space="PSUM") as mlp_ps, \
                 tc.tile_pool(name="mlp", bufs=2) as mlp:
                for c in range(BS // 512):
                    sl = slice(c * 512, (c + 1) * 512)
                    p1 = mlp_ps.tile([HID, 512], F32, tag="p1")
                    nc.tensor.matmul(p1, w1_s, xT_s[:, sl], start=True,
                                     stop=True)
                    h1a = mlp.tile([HAUG, 512], F32, tag="h1")
                    nc.sync.dma_start(out=h1a[HID:HAUG, :],
                                      in_=onesd[:, 0:512])
                    nc.scalar.activation(h1a[0:HID, :], p1, ACTF.Relu,
                                         bias=b1_s)
                    p2 = mlp_ps.tile([HID, 512], F32, tag="p2")
                    nc.tensor.matmul(p2, w2a_s, h1a, start=True, stop=True)
                    nc.scalar.activation(h2a_s[0:HID, sl], p2, ACTF.Relu)

            # ---- f32r rounded copies + residuals for the logits matmul ----
            if mm_f32:
                mm_passes = [(h2a_s, w3a_s)]
            else:
                h2r = singles.tile([HAUG, BS], F32R)
                nc.vector.tensor_copy(h2r, h2a_s)
                w3r = singles.tile([HAUG, OUT_DIM], F32R)
                nc.vector.tensor_copy(w3r, w3a_s)
                mm_passes = [(h2r, w3r)]
                if MM_MODE in ("f32r_w3res", "f32r_both"):
                    w3res = singles.tile([HAUG, OUT_DIM], F32R)
                    nc.vector.tensor_tensor(out=w3res, in0=w3a_s,
                                            in1=w3r.bitcast(F32),
                                            op=ALU.subtract)
                    mm_passes.append((h2r, w3res))
                if MM_MODE in ("f32r_h2res", "f32r_both"):
                    h2res = singles.tile([HAUG, BS], F32R)
                    nc.vector.tensor_tensor(out=h2res, in0=h2a_s,
                                            in1=h2r.bitcast(F32),
                                            op=ALU.subtract)
                    mm_passes.append((h2res, w3r))
            setup.__exit__(None, None, None)

            # ---- main loop over 16 tiles of 128 batch rows ----
            with tc.tile_pool(name="mask", bufs=2) as maskp, \
                 tc.tile_pool(name="big", bufs=3) as bigp, \
                 tc.tile_pool(name="epool", bufs=10) as epool, \
                 tc.tile_pool(name="outp", bufs=2) as outp, \
                 tc.tile_pool(name="ps", bufs=4, space="PSUM") as psp, \
                 tc.tile_pool(name="small", bufs=4) as smallp:

                dummy_l = None
                if variant == "nope":
                    dummy_l = singles.tile([128, 1024], F32)
                    nc.vector.memset(dummy_l, 0.25)

                def scatter_only_body(t):
                    rows = slice(t * 128, (t + 1) * 128)
                    m = maskp.tile([128, OUT_DIM], BF16, tag="m", name="m")
                    nc.gpsimd.local_scatter(m[:, 0:2046], ones_s,
                                            idx0_a[:, t, :], 128, 2046, K)
                    nc.gpsimd.local_scatter(m[:, 2046:4092], ones_s,
                                            idx1_a[:, t, :], 128, 2046, K)
                    nc.gpsimd.local_scatter(m[:, 4092:4096], ones_s,
                                            idx2_a[:, t, :], 128, 4, K)
                    nc.sync.dma_start(out=out[rows, 0:4].bitcast(BF16),
                                      in_=m[:, 0:8])

                def compute_phase(t):
                    rows = slice(t * 128, (t + 1) * 128)

                    m = maskp.tile([128, OUT_DIM], BF16, tag="m", name="m")
                    if do_scatter:
                        nc.gpsimd.local_scatter(m[:, 0:2046], ones_s,
                                                idx0_a[:, t, :], 128, 2046, K)
                        nc.gpsimd.local_scatter(m[:, 2046:4092], ones_s,
                                                idx1_a[:, t, :], 128, 2046, K)
                        nc.gpsimd.local_scatter(m[:, 4092:4096], ones_s,
                                                idx2_a[:, t, :], 128, 4, K)
                    else:
                        nc.vector.memset(m, 1.0)

                    # per quarter [128,1024]: matmul passes -> PSUM,
                    # mask-mult IN-PLACE in PSUM (keeps DVE off the SBUF
                    # write port it shares with the GPSIMD scatters), exp
                    # from PSUM on ACT with per-quarter accum
                    e_tiles = []
                    z_tiles = []
                    for q in range(4):
                        if variant == "nope":
                            pl = dummy_l
                        else:
                            pl = psp.tile([128, 1024], F32, tag="pl",
                                          name="pl")
                            for n in range(2):
                                ns = q * 1024 + n * 512
                                for i, (lh, rh) in enumerate(mm_passes):
                                    nc.tensor.matmul(
                                        pl[:, n * 512:(n + 1) * 512],
                                        lh[:, rows],
                                        rh[:, ns:ns + 512],
                                        start=(i == 0),
                                        stop=(i == len(mm_passes) - 1),
                                    )
                        if variant not in ("nomask",):
                            nc.vector.tensor_tensor(
                                out=pl, in0=pl,
                                in1=m[:, q * 1024:(q + 1) * 1024],
                                op=ALU.mult)
                        if variant == "noact":
                            e_tiles.append(pl)
                            z_tiles.append(None)
                            continue
                        e = epool.tile([128, 1024], F32, tag="e", name="e")
                        zp = smallp.tile([128, 1], F32, tag=f"zp{q}",
                                         name=f"zp{q}")
                        if variant == "noexp":
                            nc.scalar.activation(e, pl, ACTF.Copy)
                            nc.vector.memset(zp, 1.0)
                        else:
                            nc.scalar.activation(e, pl, ACTF.Exp,
                                                 accum_out=zp)
                        e_tiles.append(e)
                        z_tiles.append(zp)
                    return e_tiles, z_tiles

                def norm_phase(t, e_tiles, z_tiles):
                    rows = slice(t * 128, (t + 1) * 128)
                    if variant == "noact":
                        for q in range(4):
                            nc.vector.tensor_copy(
                                outp.tile([128, 1024], F32, tag="oq",
                                          name="oq"), e_tiles[q])
                        return
                    out_t = outp.tile([128, OUT_DIM], F32, tag="out",
                                      name="out_t")
                    za = smallp.tile([128, 1], F32, tag="za", name="za")
                    nc.vector.tensor_tensor(out=za, in0=z_tiles[0],
                                            in1=z_tiles[1], op=ALU.add)
                    zb = smallp.tile([128, 1], F32, tag="zb", name="zb")
                    nc.vector.tensor_tensor(out=zb, in0=z_tiles[2],
                                            in1=z_tiles[3], op=ALU.add)
                    z = smallp.tile([128, 1], F32, tag="z", name="z")
                    nc.vector.tensor_tensor(out=z, in0=za, in1=zb,
                                            op=ALU.add)
                    invz = smallp.tile([128, 1], F32, tag="invz", name="invz")
                    nc.vector.reciprocal(invz, z)
                    # normalize: 2 quarters on DVE (2x fp32), 2 on ACT
                    for q in range(2):
                        nc.vector.tensor_scalar(
                            out_t[:, q * 1024:(q + 1) * 1024], e_tiles[q],
                            invz, None, ALU.mult)
                    for q in range(2, 4):
                        nc.scalar.mul(out_t[:, q * 1024:(q + 1) * 1024],
                                      e_tiles[q], invz)
                    if variant == "nodma":
                        nc.sync.dma_start(out=out[rows, 0:8],
                                          in_=out_t[:, 0:8])
                    else:
                        nc.sync.dma_start(out=out[rows, :], in_=out_t)

                def main_loop():
                    if variant == "scatteronly":
                        for t in range(NT):
                            scatter_only_body(t)
                        return
                    if variant == "noskew":
                        for t in range(NT):
                            norm_phase(t, *compute_phase(t))
                        return
                    pending = None
                    for t in range(NT):
                        ez = compute_phase(t)
                        if pending is not None:
                            norm_phase(t - 1, *pending)
                        pending = ez
                    norm_phase(NT - 1, *pending)

                if reps == 1:
                    main_loop()
                else:
                    with tc.For_i(0, reps, 1):
                        main_loop()

    nc.compile()
    return nc


def _get_nc(reps=1, variant="full"):
    key = f"nc{reps}-{variant}-{MM_MODE}"
    if key not in _cache:
        _cache[key] = _build_nc(reps, variant)
    return _cache[key]


def _prep_inputs(x, possible_moves, W1, b1, W2, b2, W3, b3):
    x = np.ascontiguousarray(np.asarray(x, dtype=np.float32))
    pm = np.ascontiguousarray(np.asarray(possible_moves).astype(np.int32))
    W1 = np.ascontiguousarray(np.asarray(W1, dtype=np.float32))
    b1c = np.asarray(b1, dtype=np.float32).reshape(HID, 1)
    w2a = np.ascontiguousarray(
        np.concatenate([np.asarray(W2, np.float32),
                        np.asarray(b2, np.float32)[None, :]], axis=0))
    w3a = np.ascontiguousarray(
        np.concatenate([np.asarray(W3, np.float32),
                        np.asarray(b3, np.float32)[None, :]], axis=0))
    xT = np.ascontiguousarray(x.T)  # [IN_DIM, B]
    ones_row = np.ones((1, BS), np.float32)

    in_maps = []
    for c in range(NCORES):
        sl = slice(c * BS, (c + 1) * BS)
        in_maps.append({
            "xT": np.ascontiguousarray(xT[:, sl]),
            "pm": np.ascontiguousarray(pm[sl, :]),
            "w1": W1,
            "b1": b1c,
            "w2a": w2a,
            "w3a": w3a,
            "onesd": ones_row,
        })
    return in_maps


def kernel(x, possible_moves, W1, b1, W2, b2, W3, b3):
    from concourse.bass_utils import run_bass_kernel_spmd

    in_maps = _prep_inputs(x, possible_moves, W1, b1, W2, b2, W3, b3)
    nc = _get_nc()
    res = run_bass_kernel_spmd(nc, in_maps, core_ids=list(range(NCORES)))
    return np.concatenate([res.results[c]["out"] for c in range(NCORES)],
                          axis=0)


# revision 36
# speedup vs baseline: 5.2433x; 1.0369x over previous
"""TRN2 Bass kernel for nn_DQN (topk_masking).

reference:
    h = relu(x @ W1 + b1); h = relu(h @ W2 + b2); logits = h @ W3 + b3
    mask[b, possible_moves[b, :]] = 1
    out = softmax(logits * mask, axis=1)

Strategy (8 NeuronCores, data-parallel over batch, 2048 rows/core):
  - host: transpose x -> xT [128, B]; fold b2/b3 into augmented weight rows.
  - index prep first: pm chunks + 5 batched DVE transforms per chunk so the
    GPSIMD scatter stream starts within a few us and never waits on DVE.
  - tiny MLP on PE in fp32 (cheap); the big logits matmul runs as float32r
    (1 cyc/row vs 4 for fp32) with f32r *residual correction passes*
    accumulated in PSUM: logits = h2_r @ W3_r (+ h2_r @ W3res_r + h2res_r @
    W3_r per MM_MODE), where X_r = round_f32r(X) comes from the engines' own
    f32r write-rounding and Xres_r = round_f32r(X - X_r).
  - GPSIMD local_scatter builds the 0/1 mask [128 rows, 4096] bf16 per tile
    (3 scatters: [0,2046)/[2046,4092)/[4092,4096), num_elems cap is 2046;
    out-of-range indices are made negative = ignored by the ucode).
  - DVE: filtered = logits * mask; ACT: E = exp(filtered) with accum_out
    giving the row-sum Z directly (illegal positions contribute exp(0)=1,
    matching the reference); out = E * (1/Z) split DVE/ACT.

reps>1 wraps the main loop in a dynamic For_i purely for timing experiments.
"""

import os
import sys

import numpy as np

for _p in ("/root/.axon_site", "/root/.axon_site/_ro/trn_rl_repo",
           "/root/.axon_site/_ro/pypackages"):
    if os.path.isdir(_p) and _p not in sys.path:
        sys.path.append(_p)

B, IN_DIM, HID, OUT_DIM, K = 16384, 128, 24, 4096, 256
NCORES = 8
BS = B // NCORES          # 2048 rows per core
NT = BS // 128            # 16 tiles of 128 rows
HAUG = HID + 1            # 25: hidden + ones row

_cache = {}

# logits matmul precision mode:
#   "f32"         exact, 4 cyc/row on PE (slow)
#   "f32r"        single f32r pass (~1.3e-4 rel err)
#   "f32r_w3res"  f32r + W3-residual pass
#   "f32r_h2res"  f32r + h2-residual pass
#   "f32r_both"   f32r + both residual passes
MM_MODE = "f32r_both"


def _build_nc(reps=1, variant="full"):
    import concourse.bacc as bacc
    import concourse.mybir as mybir
    import concourse.tile as tile

    F32 = mybir.dt.float32
    F32R = mybir.dt.float32r
    BF16 = mybir.dt.bfloat16
    I32 = mybir.dt.int32
    I16 = mybir.dt.int16
    ALU = mybir.AluOpType
    ACTF = mybir.ActivationFunctionType

    nc = bacc.Bacc("TRN2", target_bir_lowering=False, debug=False,
                   num_devices=NCORES)

    xT = nc.dram_tensor("xT", [IN_DIM, BS], F32, kind="ExternalInput").ap()
    pm = nc.dram_tensor("pm", [BS, K], I32, kind="ExternalInput").ap()
    w1 = nc.dram_tensor("w1", [IN_DIM, HID], F32, kind="ExternalInput").ap()
    b1 = nc.dram_tensor("b1", [HID, 1], F32, kind="ExternalInput").ap()
    w2a = nc.dram_tensor("w2a", [HAUG, HID], F32, kind="ExternalInput").ap()
    w3a = nc.dram_tensor("w3a", [HAUG, OUT_DIM], F32,
                         kind="ExternalInput").ap()
    onesd = nc.dram_tensor("onesd", [1, BS], F32, kind="ExternalInput").ap()
    out = nc.dram_tensor("out", [BS, OUT_DIM], F32, kind="ExternalOutput").ap()

    GRP = 4           # tiles per index-transform group
    NGRP = NT // GRP
    mm_f32 = (MM_MODE == "f32")

    with tile.TileContext(nc) as tc:
        with tc.tile_pool(name="singles", bufs=1) as singles:
            do_scatter = variant in ("full", "noexp", "noskew", "scatteronly")

            pmv = pm.rearrange("(t p) k -> p t k", p=128)
            if do_scatter:
                idx0_a = singles.tile([128, NT, K], I16)
                idx1_a = singles.tile([128, NT, K], I16)
                idx2_a = singles.tile([128, NT, K], I16)
                with tc.tile_pool(name="pmp", bufs=2) as pmpool:
                    for g in range(NGRP):
                        gs = slice(g * GRP, (g + 1) * GRP)
                        pm_g = pmpool.tile([128, GRP, K], I32, tag="pm",
                                           name="pm_g")
                        nc.scalar.dma_start(out=pm_g, in_=pmv[:, gs, :])
                        v0 = pmpool.tile([128, GRP, K], I16, tag="v0",
                                         name="v0")
                        nc.vector.tensor_scalar(v0, pm_g, 2046, None,
                                                ALU.is_lt)
                        nc.vector.affine_then_add(idx0_a[:, gs, :], v0,
                                                  pm_g, 4096.0, -4096.0)
                        v1 = pmpool.tile([128, GRP, K], I16, tag="v1",
                                         name="v1")
                        nc.vector.tensor_scalar(v1, pm_g, 4092, None,
                                                ALU.is_lt)
                        nc.vector.affine_then_add(idx1_a[:, gs, :], v1,
                                                  pm_g, 2050.0, -4096.0)
                        nc.vector.tensor_scalar(idx2_a[:, gs, :], pm_g,
                                                4092, None, ALU.subtract)

            ones_s = singles.tile([128, K], BF16)
            nc.vector.memset(ones_s, 1.0)

            # setup pool: fp32 masters that die once the f32r copies exist
            # (in f32 mode h2a/w3a live on in singles instead)
            setup = tc.tile_pool(name="setup", bufs=1)
            setupp = setup.__enter__()
            big_pool = singles if mm_f32 else setupp
            xT_s = setupp.tile([IN_DIM, BS], F32, name="xT_s")
            nc.sync.dma_start(out=xT_s, in_=xT)
            w1_s = setupp.tile([IN_DIM, HID], F32, name="w1_s")
            nc.sync.dma_start(out=w1_s, in_=w1)
            b1_s = setupp.tile([HID, 1], F32, name="b1_s")
            nc.sync.dma_start(out=b1_s, in_=b1)
            w2a_s = setupp.tile([HAUG, HID], F32, name="w2a_s")
            nc.sync.dma_start(out=w2a_s, in_=w2a)
            w3a_s = big_pool.tile([HAUG, OUT_DIM], F32, name="w3a_s")
            nc.sync.dma_start(out=w3a_s, in_=w3a)
            # ones row (partition 24): engines can't target base partition 24
            # alone -> DMA the row from a host-side ones constant; relu
            # writes rows 0..23
            h2a_s = big_pool.tile([HAUG, BS], F32, name="h2a_s")
            nc.sync.dma_start(out=h2a_s[HID:HAUG, :], in_=onesd)

            # W3 f32r rounding is MLP-independent: do it up front, with
            # the big copy on ACT (idle here) rather than DVE
            if not mm_f32:
                w3r = singles.tile([HAUG, OUT_DIM], F32R, name="w3r")
                nc.scalar.activation(w3r, w3a_s, ACTF.Copy)
                if MM_MODE in ("f32r_w3res", "f32r_both"):
                    w3res = singles.tile([HAUG, OUT_DIM], F32R, name="w3res")
                    nc.vector.tensor_tensor(out=w3res, in0=w3a_s,
                                            in1=w3r.bitcast(F32),
                                            op=ALU.subtract)

            # ---- tiny MLP (fp32): h2aug [25, BS] in 512-col chunks ----
            with tc.tile_pool(name="mlp_ps", bufs=2, space="PSUM") as mlp_ps, \
                 tc.tile_pool(name="mlp", bufs=2) as mlp:
                for c in range(BS // 512):
                    sl = slice(c * 512, (c + 1) * 512)
                    p1 = mlp_ps.tile([HID, 512], F32, tag="p1")
                    nc.tensor.matmul(p1, w1_s, xT_s[:, sl], start=True,
                                     stop=True)
                    h1a = mlp.tile([HAUG, 512], F32, tag="h1")
                    nc.sync.dma_start(out=h1a[HID:HAUG, :],
                                      in_=onesd[:, 0:512])
                    nc.scalar.activation(h1a[0:HID, :], p1, ACTF.Relu,
                                         bias=b1_s)
                    p2 = mlp_ps.tile([HID, 512], F32, tag="p2")
                    nc.tensor.matmul(p2, w2a_s, h1a, start=True, stop=True)
                    nc.scalar.activation(h2a_s[0:HID, sl], p2, ACTF.Relu)

            # ---- h2 f32r rounding + residuals ----
            if mm_f32:
                mm_passes = [(h2a_s, w3a_s)]
            else:
                h2r = singles.tile([HAUG, BS], F32R)
                nc.scalar.activation(h2r, h2a_s, ACTF.Copy)
                mm_passes = [(h2r, w3r)]
                if MM_MODE in ("f32r_w3res", "f32r_both"):
                    mm_passes.append((h2r, w3res))
                if MM_MODE in ("f32r_h2res", "f32r_both"):
                    h2res = singles.tile([HAUG, BS], F32R)
                    nc.vector.tensor_tensor(out=h2res, in0=h2a_s,
                                            in1=h2r.bitcast(F32),
                                            op=ALU.subtract)
                    mm_passes.append((h2res, w3r))
            setup.__exit__(None, None, None)

            # ---- main loop over 16 tiles of 128 batch rows ----
            with tc.tile_pool(name="mask", bufs=2) as maskp, \
                 tc.tile_pool(name="big", bufs=3) as bigp, \
                 tc.tile_pool(name="epool", bufs=10) as epool, \
                 tc.tile_pool(name="outp", bufs=2) as outp, \
                 tc.tile_pool(name="ps", bufs=4, space="PSUM") as psp, \
                 tc.tile_pool(name="small", bufs=4) as smallp:

                dummy_l = None
                if variant == "nope":
                    dummy_l = singles.tile([128, 1024], F32)
                    nc.vector.memset(dummy_l, 0.25)

                def scatter_only_body(t):
                    rows = slice(t * 128, (t + 1) * 128)
                    m = maskp.tile([128, OUT_DIM], BF16, tag="m", name="m")
                    nc.gpsimd.local_scatter(m[:, 0:2046], ones_s,
                                            idx0_a[:, t, :], 128, 2046, K)
                    nc.gpsimd.local_scatter(m[:, 2046:4092], ones_s,
                                            idx1_a[:, t, :], 128, 2046, K)
                    nc.gpsimd.local_scatter(m[:, 4092:4096], ones_s,
                                            idx2_a[:, t, :], 128, 4, K)
                    nc.sync.dma_start(out=out[rows, 0:4].bitcast(BF16),
                                      in_=m[:, 0:8])

                def compute_phase(t):
                    rows = slice(t * 128, (t + 1) * 128)

                    m = maskp.tile([128, OUT_DIM], BF16, tag="m", name="m")
                    if do_scatter:
                        nc.gpsimd.local_scatter(m[:, 0:2046], ones_s,
                                                idx0_a[:, t, :], 128, 2046, K)
                        nc.gpsimd.local_scatter(m[:, 2046:4092], ones_s,
                                                idx1_a[:, t, :], 128, 2046, K)
                        nc.gpsimd.local_scatter(m[:, 4092:4096], ones_s,
                                                idx2_a[:, t, :], 128, 4, K)
                    else:
                        nc.vector.memset(m, 1.0)

                    # per quarter [128,1024]: matmul passes -> PSUM,
                    # mask-mult IN-PLACE in PSUM (keeps DVE off the SBUF
                    # write port it shares with the GPSIMD scatters), exp
                    # from PSUM on ACT with per-quarter accum
                    e_tiles = []
                    z_tiles = []
                    for q in range(4):
                        if variant == "nope":
                            pl = dummy_l
                        else:
                            pl = psp.tile([128, 1024], F32, tag="pl",
                                          name="pl")
                            for n in range(2):
                                ns = q * 1024 + n * 512
                                for i, (lh, rh) in enumerate(mm_passes):
                                    nc.tensor.matmul(
                                        pl[:, n * 512:(n + 1) * 512],
                                        lh[:, rows],
                                        rh[:, ns:ns + 512],
                                        start=(i == 0),
                                        stop=(i == len(mm_passes) - 1),
                                    )
                        if variant not in ("nomask",):
                            nc.vector.tensor_tensor(
                                out=pl, in0=pl,
                                in1=m[:, q * 1024:(q + 1) * 1024],
                                op=ALU.mult)
                        if variant == "noact":
                            e_tiles.append(pl)
                            z_tiles.append(None)
                            continue
                        e = epool.tile([128, 1024], F32, tag="e", name="e")
                        zp = smallp.tile([128, 1], F32, tag=f"zp{q}",
                                         name=f"zp{q}")
                        if variant == "noexp":
                            nc.scalar.activation(e, pl, ACTF.Copy)
                            nc.vector.memset(zp, 1.0)
                        else:
                            nc.scalar.activation(e, pl, ACTF.Exp,
                                                 accum_out=zp)
                        e_tiles.append(e)
                        z_tiles.append(zp)
                    return e_tiles, z_tiles

                def norm_phase(t, e_tiles, z_tiles):
                    rows = slice(t * 128, (t + 1) * 128)
                    if variant == "noact":
                        for q in range(4):
                            nc.vector.tensor_copy(
                                outp.tile([128, 1024], F32, tag="oq",
                                          name="oq"), e_tiles[q])
                        return
                    out_t = outp.tile([128, OUT_DIM], F32, tag="out",
                                      name="out_t")
                    za = smallp.tile([128, 1], F32, tag="za", name="za")
                    nc.vector.tensor_tensor(out=za, in0=z_tiles[0],
                                            in1=z_tiles[1], op=ALU.add)
                    zb = smallp.tile([128, 1], F32, tag="zb", name="zb")
                    nc.vector.tensor_tensor(out=zb, in0=z_tiles[2],
                                            in1=z_tiles[3], op=ALU.add)
                    z = smallp.tile([128, 1], F32, tag="z", name="z")
                    nc.vector.tensor_tensor(out=z, in0=za, in1=zb,
                                            op=ALU.add)
                    invz = smallp.tile([128, 1], F32, tag="invz", name="invz")
                    nc.vector.reciprocal(invz, z)
                    # normalize: 2 quarters on DVE (2x fp32), 2 on ACT
                    for q in range(2):
                        nc.vector.tensor_scalar(
                            out_t[:, q * 1024:(q + 1) * 1024], e_tiles[q],
                            invz, None, ALU.mult)
                    for q in range(2, 4):
                        nc.scalar.mul(out_t[:, q * 1024:(q + 1) * 1024],
                                      e_tiles[q], invz)
                    if variant == "nodma":
                        nc.sync.dma_start(out=out[rows, 0:8],
                                          in_=out_t[:, 0:8])
                    else:
                        nc.sync.dma_start(out=out[rows, :], in_=out_t)

                def main_loop():
                    if variant == "scatteronly":
                        for t in range(NT):
                            scatter_only_body(t)
                        return
                    if variant == "noskew":
                        for t in range(NT):
                            norm_phase(t, *compute_phase(t))
                        return
                    pending = None
                    for t in range(NT):
                        ez = compute_phase(t)
                        if pending is not None:
                            norm_phase(t - 1, *pending)
                        pending = ez
                    norm_phase(NT - 1, *pending)

                if reps == 1:
                    main_loop()
                else:
                    with tc.For_i(0, reps, 1):
                        main_loop()

    nc.compile()
    return nc


def _get_nc(reps=1, variant="full"):
    key = f"nc{reps}-{variant}-{MM_MODE}"
    if key not in _cache:
        _cache[key] = _build_nc(reps, variant)
    return _cache[key]


def _prep_inputs(x, possible_moves, W1, b1, W2, b2, W3, b3):
    x = np.ascontiguousarray(np.asarray(x, dtype=np.float32))
    pm = np.ascontiguousarray(np.asarray(possible_moves).astype(np.int32))
    W1 = np.ascontiguousarray(np.asarray(W1, dtype=np.float32))
    b1c = np.asarray(b1, dtype=np.float32).reshape(HID, 1)
    w2a = np.ascontiguousarray(
        np.concatenate([np.asarray(W2, np.float32),
                        np.asarray(b2, np.float32)[None, :]], axis=0))
    w3a = np.ascontiguousarray(
        np.concatenate([np.asarray(W3, np.float32),
                        np.asarray(b3, np.float32)[None, :]], axis=0))
    xT = np.ascontiguousarray(x.T)  # [IN_DIM, B]
    ones_row = np.ones((1, BS), np.float32)

    in_maps = []
    for c in range(NCORES):
        sl = slice(c * BS, (c + 1) * BS)
        in_maps.append({
            "xT": np.ascontiguousarray(xT[:, sl]),
            "pm": np.ascontiguousarray(pm[sl, :]),
            "w1": W1,
            "b1": b1c,
            "w2a": w2a,
            "w3a": w3a,
            "onesd": ones_row,
        })
    return in_maps


def kernel(x, possible_moves, W1, b1, W2, b2, W3, b3):
    from concourse.bass_utils import run_bass_kernel_spmd

    in_maps = _prep_inputs(x, possible_moves, W1, b1, W2, b2, W3, b3)
    nc = _get_nc()
    res = run_bass_kernel_spmd(nc, in_maps, core_ids=list(range(NCORES)))
    return np.concatenate([res.results[c]["out"] for c in range(NCORES)],
                          axis=0)
